# revision 1
# baseline (speedup 1.0000x reference)
"""Dilated (dil=2) 7x7 window self-attention, 4 heads x 32 dim, on 8 trn2 cores.

Strategy: spatial sharding over image rows (12 rows/core, 6-row halo).
Inside each core, the dilation-2 window decomposes the image into 4
cosets (row/col parity); within a coset the attention is a dense 7x7
window on a 48x48 grid.  All tensors are kept channel-major [128, pix];
logits are computed transposed [nk, nq] per (batch, coset) block so both
attention einsums are matmuls without any transposes:

  K^T Q  : 16-tile-packed 32x32 matmuls (per-head, reduction over d=32)
  softmax: unnormalized exp (no max-subtraction; logits are tiny) with
           the mask bias (-60 per masked key pixel) folded into the ACT
           exp bias; out-of-window pairs zeroed by one elementwise mul
           with a precomputed 0/1 window tensor; the softmax denominator
           comes from an extra ones-weight matmul pass and is divided
           out after attn@V.
  attn@V : col-tiled (4 heads) matmuls, reduction over nk chunks of 96,
           V produced directly in transposed [pix, ch] form by swapping
           the matmul operands of the V projection.
"""

import numpy as np

HEADS, D, WIN, DIL = 4, 32, 7, 2
B, C, H, W = 2, 128, 96, 96
CORES, RPC = 8, 12
CR, KR, W2 = 6, 12, 48            # coset query rows / key rows (halo) / cols
NQ, NK = CR * W2, KR * W2         # 288, 576
NBLK = B * 4                      # (batch, coset) blocks per core
SCALE = float(1.0 / np.sqrt(D))
MBIAS = -60.0

_prog = None


def _band32(c):
    """query-row band of 32-pixel key subchunk c (inclusive lo, hi)."""
    r_lo, r_hi = (32 * c) // W2, (32 * c + 31) // W2
    lo = max(0, r_lo - 6)
    hi = min(CR - 1, r_hi)
    return lo, hi


def _band(g):
    """query-row band of key-row pair {2g, 2g+1}: inclusive (lo, hi)."""
    rows = [i for i in range(CR)
            if (i <= 2 * g <= i + 6) or (i <= 2 * g + 1 <= i + 6)]
    return rows[0], rows[-1]


def _win_mask():
    """[NK, NQ] 0/1 in-window mask for one (batch, coset) block."""
    rr = np.arange(KR)[:, None, None, None]
    cc = np.arange(W2)[None, :, None, None]
    ii = np.arange(CR)[None, None, :, None]
    jj = np.arange(W2)[None, None, None, :]
    win = ((rr - ii >= 0) & (rr - ii <= 6) & (np.abs(cc - jj) <= 3))
    return win.reshape(NK, NQ).astype(np.float32)


def _build_program():
    import concourse.bass as bass
    import concourse.tile as tile
    from concourse import mybir

    nc = bass.Bass("TRN2", target_bir_lowering=False, debug=False,
                   num_devices=CORES)
    f32 = mybir.dt.float32
    mdt = mybir.dt.float32
    xc = nc.dram_tensor("xc", [128, NBLK * NK], f32, kind="ExternalInput").ap()
    mb_i = nc.dram_tensor("mb", [128, NBLK * 6], mybir.dt.int32,
                          kind="ExternalInput").ap()
    winm = nc.dram_tensor("winm", [128, 4 * 6 * NQ], f32,
                          kind="ExternalInput").ap()
    wq = nc.dram_tensor("wq", [128, 128], f32, kind="ExternalInput").ap()
    wk = nc.dram_tensor("wk", [128, 128], f32, kind="ExternalInput").ap()
    wv = nc.dram_tensor("wv", [128, 128], f32, kind="ExternalInput").ap()
    wp = nc.dram_tensor("wp", [128, 128], f32, kind="ExternalInput").ap()
    out = nc.dram_tensor("out", [128, NBLK * NQ], f32,
                         kind="ExternalOutput").ap()

    with tile.TileContext(nc) as tc:
        with tc.tile_pool(name="cst", bufs=1) as cst, \
             tc.tile_pool(name="big", bufs=1) as big, \
             tc.tile_pool(name="qk", bufs=1) as qkp, \
             tc.tile_pool(name="vt", bufs=2) as vtp, \
             tc.tile_pool(name="att", bufs=2) as attp, \
             tc.tile_pool(name="oev", bufs=3) as oev, \
             tc.tile_pool(name="psL", bufs=1, space="PSUM") as psL, \
             tc.tile_pool(name="psO", bufs=1, space="PSUM") as psO, \
             tc.tile_pool(name="psP", bufs=2, space="PSUM") as psP:

            w_q = cst.tile([128, 128], mdt)
            nc.gpsimd.dma_start(out=w_q[:], in_=wq[:])
            w_k = cst.tile([128, 128], mdt)
            nc.gpsimd.dma_start(out=w_k[:], in_=wk[:])
            w_v = cst.tile([128, 128], mdt)
            nc.gpsimd.dma_start(out=w_v[:], in_=wv[:])
            w_p = cst.tile([128, 128], mdt)
            nc.gpsimd.dma_start(out=w_p[:], in_=wp[:])

            X = big.tile([128, NBLK * NK], mdt)
            nc.gpsimd.dma_start(out=X[:], in_=xc[:])
            WM = big.tile([128, 4 * 6 * NQ], f32)   # win mask, coset-major
            nc.gpsimd.dma_start(out=WM[:], in_=winm[:])

            mbi = cst.tile([128, NBLK * 6], f32)
            mbraw = cst.tile([128, NBLK * 6], mybir.dt.int32)
            nc.gpsimd.dma_start(out=mbraw[:], in_=mb_i[:])
            nc.vector.tensor_copy(mbi[:], mbraw[:])        # int -> float
            mbias = cst.tile([128, NBLK * 6], f32)
            nc.vector.tensor_scalar(
                out=mbias[:], in0=mbi[:], scalar1=-MBIAS, scalar2=MBIAS,
                op0=mybir.AluOpType.mult, op1=mybir.AluOpType.add,
            )  # m*60 - 60 -> 0 (keep) / -60 (masked)

            pL0 = psL.tile([128, 2048], f32, tag="psL")
            nc.vector.memset(pL0[:], 0.0)

            ones_f = cst.tile([128, 32], f32)
            nc.vector.memset(ones_f[:], 1.0)
            ones = cst.tile([128, 32], mdt)
            nc.vector.tensor_copy(ones[:], ones_f[:])

            # Q and K channel-major projections for all blocks.
            Q = qkp.tile([128, NBLK * NQ], mdt)
            K = qkp.tile([128, NBLK * NK], mdt)
            for blk in range(NBLK):
                pq = psP.tile([128, 512], f32, tag="psP")
                nc.tensor.matmul(out=pq[:, :NQ], lhsT=w_q[:],
                                 rhs=X[:, blk * NK + 144: blk * NK + 144 + NQ],
                                 start=True, stop=True)
                if blk % 2:
                    nc.scalar.copy(out=Q[:, blk * NQ:(blk + 1) * NQ], in_=pq[:, :NQ])
                else:
                    nc.vector.tensor_copy(Q[:, blk * NQ:(blk + 1) * NQ], pq[:, :NQ])
                for half in range(2):
                    pk = psP.tile([128, 512], f32, tag="psP")
                    sl = slice(blk * NK + half * NQ, blk * NK + (half + 1) * NQ)
                    nc.tensor.matmul(out=pk[:, :NQ], lhsT=w_k[:], rhs=X[:, sl],
                                     start=True, stop=True)
                    if half:
                        nc.scalar.copy(out=K[:, sl], in_=pk[:, :NQ])
                    else:
                        nc.vector.tensor_copy(K[:, sl], pk[:, :NQ])

            for blk in range(NBLK):
                cs = blk % 4
                # --- V^T production: 6 chunks of 96 pixels ---
                VT = vtp.tile([128, 6 * 128], mdt, tag="vt")
                for pair in range(3):       # two 96-chunks per psum bank
                    pv = psP.tile([128, 512], f32, tag="psP")
                    for k2 in range(2):
                        g = pair * 2 + k2
                        nc.tensor.matmul(
                            out=pv[:96, k2 * 128:(k2 + 1) * 128],
                            lhsT=X[:, blk * NK + 96 * g:
                                   blk * NK + 96 * (g + 1)],
                            rhs=w_v[:], start=True, stop=True)
                    if pair % 2:
                        nc.scalar.copy(out=VT[:96, pair * 256:(pair + 1) * 256],
                                       in_=pv[:96, :256])
                    else:
                        nc.vector.tensor_copy(VT[:96, pair * 256:(pair + 1) * 256],
                                              pv[:96, :256])

                # --- phase 1 + exp + window mask ---
                attnT = attp.tile([128, 4 * 6 * NQ], mdt, tag="att")
                for g in range(6):
                    lo, hi = _band(g)
                    nlo, nn = lo * W2, (hi - lo + 1) * W2
                    pL = psL.tile([128, 2048], f32, tag="psL")
                    for k3 in range(3):
                        c32 = 3 * g + k3
                        lo3, hi3 = _band32(c32)
                        n3, nn3 = lo3 * W2, (hi3 - lo3 + 1) * W2
                        for h in range(4):
                            nc.tensor.matmul(
                                out=pL[32 * k3:32 * k3 + 32,
                                       512 * h + n3: 512 * h + n3 + nn3],
                                lhsT=K[32 * h:32 * h + 32,
                                       blk * NK + 32 * c32:
                                       blk * NK + 32 * c32 + 32].bitcast(f32),
                                rhs=Q[32 * h:32 * h + 32,
                                      blk * NQ + n3:
                                      blk * NQ + n3 + nn3].bitcast(f32),
                                start=True, stop=True,
                                tile_position=(32 * h, 32 * k3),
                            )
                    # exp over 4 heads at once: AP [96, (4 banks, nn)]
                    src = pL[:96].rearrange("p (h n) -> p h n", h=4)[:, :, nlo:nlo + nn]
                    dst = attnT[:96].rearrange("p (h g n) -> p h g n", h=4, g=6)[:, :, g, nlo:nlo + nn]
                    nc.scalar.activation(
                        out=dst, in_=src,
                        func=mybir.ActivationFunctionType.Exp,
                        bias=mbias[0:96, blk * 6 + g: blk * 6 + g + 1],
                        scale=SCALE,
                    )
                    # zero out-of-window pairs (win==0) and garbage rows
                    wsrc = WM[0:96, cs * 6 * NQ + g * NQ + nlo:
                              cs * 6 * NQ + g * NQ + nlo + nn]
                    for h in range(4):
                        dsth = attnT[0:96, (h * 6 + g) * NQ + nlo:
                                     (h * 6 + g) * NQ + nlo + nn]
                        eng = nc.vector if h % 2 else nc.gpsimd
                        eng.tensor_mul(out=dsth, in0=dsth, in1=wsrc)

                # --- phase 2 (attn @ V^T) + rowsum, col-tiled by head ---
                pO = psO.tile([128, 512], f32, tag="psO")
                pS = psO.tile([128, 512], f32, tag="psS")
                for g in range(6):
                    lo, hi = _band(g)
                    nlo, nn = lo * W2, (hi - lo + 1) * W2
                    for h in range(4):
                        rhs = attnT[0:96, (h * 6 + g) * NQ + nlo:
                                    (h * 6 + g) * NQ + nlo + nn].bitcast(f32)
                        nc.tensor.matmul(
                            out=pO[32 * h:32 * h + 32, nlo:nlo + nn],
                            lhsT=VT[0:96, g * 128 + 32 * h:
                                    g * 128 + 32 * h + 32].bitcast(f32),
                            rhs=rhs, start=(g == 0), stop=(g == 5),
                            tile_position=(0, 32 * h),
                        )
                        nc.tensor.matmul(
                            out=pS[32 * h:32 * h + 32, nlo:nlo + nn],
                            lhsT=ones[0:96, :].bitcast(f32),
                            rhs=rhs, start=(g == 0), stop=(g == 5),
                            tile_position=(0, 32 * h),
                        )
                rcp = oev.tile([128, NQ], f32, tag="rcp")
                nc.vector.reciprocal(out=rcp[:], in_=pS[:, :NQ])
                onrm = oev.tile([128, NQ], mdt, tag="onrm")
                nc.vector.tensor_mul(out=onrm[:], in0=pO[:, :NQ], in1=rcp[:])

                # --- final projection ---
                pF = psP.tile([128, 512], f32, tag="psP")
                nc.tensor.matmul(out=pF[:, :NQ], lhsT=w_p[:], rhs=onrm[:],
                                 start=True, stop=True)
                osb = oev.tile([128, NQ], f32, tag="osb")
                nc.scalar.copy(out=osb[:], in_=pF[:, :NQ])
                nc.gpsimd.dma_start(out=out[:, blk * NQ:(blk + 1) * NQ],
                                    in_=osb[:])

    _split_multi_waits(nc)
    return nc


def _split_multi_waits(nc):
    """This walrus build rejects >1 sem wait per instruction: move extra
    waits onto dedicated single-wait NoOps inserted just before."""
    import copy
    from concourse import mybir

    tmpl = nc.sync.nop(nofuse=True, hint="wsplit_template").ins
    bb0 = nc.cur_bb.bb
    bb0.instructions = [i for i in bb0.instructions if i.name != tmpl.name]
    tmpl = copy.deepcopy(tmpl)

    ctr = 0
    for f in nc.m.functions:
        for bb in f.blocks:
            insts = list(bb.instructions)
            new, changed = [], False
            for inst in insts:
                si = getattr(inst, "sync_info", None)
                waits = list(si.on_wait) if si is not None and si.on_wait else []
                if len(waits) > 1:
                    for w in waits[:-1]:
                        ctr += 1
                        nop = copy.deepcopy(tmpl)
                        nop.name = f"I-wsplit{ctr}"
                        nop.engine = inst.engine
                        nop.sync_info = mybir.SyncInfo(on_wait=[w], on_update=[])
                        new.append(nop)
                    si.on_wait = [waits[-1]]
                    changed = True
                new.append(inst)
            if changed:
                bb.instructions = new


def _host_prep(x, m):
    xs, ms = [], []
    for k in range(CORES):
        r0 = 12 * k - 6
        xpad = np.zeros((B, C, 24, W), np.float32)
        mpad = np.zeros((B, 1, 24, W), np.int32)
        lo, hi = max(0, r0), min(H, r0 + 24)
        xpad[:, :, lo - r0:hi - r0] = x[:, :, lo:hi]
        mpad[:, :, lo - r0:hi - r0] = m[:, :, lo:hi]
        xcs = xpad.reshape(B, C, KR, 2, W2, 2).transpose(1, 0, 3, 5, 2, 4)
        xcs = np.ascontiguousarray(xcs.reshape(C, NBLK * NK))
        mc = mpad.reshape(B, 1, KR, 2, W2, 2).transpose(1, 0, 3, 5, 2, 4)
        mc = mc.reshape(B, 4, NK)
        mb = np.ones((128, NBLK * 6), np.int32)
        for b in range(B):
            for cspar in range(4):
                for g in range(6):
                    mb[:96, (b * 4 + cspar) * 6 + g] = \
                        mc[b, cspar, 96 * g:96 * (g + 1)]
        xs.append(xcs)
        ms.append(np.ascontiguousarray(mb))
    return xs, ms


def _host_win():
    """[128, 4*6*NQ]: win mask in attnT layout (heads=4 share; here the
    '4' axis is heads, identical; partitions 96-127 zero)."""
    win = _win_mask()                        # [NK, NQ]
    wm = np.zeros((128, 4, 6, NQ), np.float32)
    for g in range(6):
        wm[:96, :, g, :] = win[96 * g:96 * (g + 1), None, :]
    return np.ascontiguousarray(wm.reshape(128, 4 * 6 * NQ))


def kernel(x, m, Wq, Wk, Wv, Wp):
    global _prog
    from concourse.bass_utils import run_bass_kernel_spmd

    x = np.asarray(x, dtype=np.float32)
    m = np.asarray(m, dtype=np.int32)
    if _prog is None:
        _prog = _build_program()
    nc = _prog

    xs, ms = _host_prep(x, m)
    wmask = _host_win()
    base = {
        "winm": wmask,
        "wq": np.ascontiguousarray(np.asarray(Wq, np.float32).T),
        "wk": np.ascontiguousarray(np.asarray(Wk, np.float32).T),
        "wv": np.ascontiguousarray(np.asarray(Wv, np.float32).T),
        "wp": np.ascontiguousarray(np.asarray(Wp, np.float32).T),
    }
    in_maps = [{**base, "xc": xs[k], "mb": ms[k]} for k in range(CORES)]
    res = run_bass_kernel_spmd(nc, in_maps, list(range(CORES)))

    full = np.zeros((B, C, H, W), np.float32)
    for k in range(CORES):
        oc = res.results[k]["out"].reshape(C, B, 2, 2, CR, W2)
        o = oc.transpose(1, 0, 4, 2, 5, 3).reshape(B, C, 12, 96)
        full[:, :, 12 * k:12 * k + 12, :] = o
    return full



# revision 4
# speedup vs baseline: 1.6549x; 1.6549x over previous
"""Dilated (dil=2) 7x7 window self-attention, 4 heads x 32 dim, on 8 trn2 cores.

Strategy: spatial sharding over image rows (12 rows/core, 6-row halo).
Inside each core, the dilation-2 window decomposes the image into 4
cosets (row/col parity); within a coset the attention is a dense 7x7
window on a 48x48 grid.  All tensors are kept channel-major [128, pix];
logits are computed transposed [nk, nq] per (batch, coset) block so both
attention einsums are matmuls without any transposes.

All matmuls run in bf16 (1 PE cycle/row vs 4 for fp32), accumulation in
fp32 PSUM.  Host pre-casts x and the weights to bf16.

  K^T Q  : per (g, head) one [32,96]-lhsT matmul covering the whole
           96-key chunk (full-partition output), banded over query rows.
  softmax: unnormalized exp (no max-subtraction; logits are tiny) with
           the key-pixel mask bias (-60 per masked key) folded into the
           ACT exp bias, output directly in bf16; out-of-window pairs
           zeroed by elementwise mul with a precomputed 0/1 window
           tensor (vector engine, bf16 2x mode); the softmax denominator
           comes from a ones-weight matmul pass and is divided out after
           attn@V.
  attn@V : col-tiled (4 heads) matmuls, reduction over nk chunks of 96,
           V produced directly in transposed [pix, ch] form by swapping
           the matmul operands of the V projection.
"""

import numpy as np

HEADS, D, WIN, DIL = 4, 32, 7, 2
B, C, H, W = 2, 128, 96, 96
CORES, RPC = 8, 12
CR, KR, W2 = 6, 12, 48            # coset query rows / key rows (halo) / cols
NQ, NK = CR * W2, KR * W2         # 288, 576
NBLK = B * 4                      # (batch, coset) blocks per core
SCALE = float(1.0 / np.sqrt(D))
MBIAS = -60.0

_prog = None


def _band(g):
    """query-row band of key-row pair {2g, 2g+1}: inclusive (lo, hi)."""
    rows = [i for i in range(CR)
            if (i <= 2 * g <= i + 6) or (i <= 2 * g + 1 <= i + 6)]
    return rows[0], rows[-1]


def _win_mask():
    """[NK, NQ] 0/1 in-window mask for one (batch, coset) block."""
    rr = np.arange(KR)[:, None, None, None]
    cc = np.arange(W2)[None, :, None, None]
    ii = np.arange(CR)[None, None, :, None]
    jj = np.arange(W2)[None, None, None, :]
    win = ((rr - ii >= 0) & (rr - ii <= 6) & (np.abs(cc - jj) <= 3))
    return win.reshape(NK, NQ).astype(np.float32)


def _build_program():
    import concourse.bass as bass
    import concourse.tile as tile
    from concourse import mybir

    nc = bass.Bass("TRN2", target_bir_lowering=False, debug=False,
                   num_devices=CORES)
    f32 = mybir.dt.float32
    bf = mybir.dt.bfloat16
    xc = nc.dram_tensor("xc", [128, NBLK * NK], bf, kind="ExternalInput").ap()
    mb_i = nc.dram_tensor("mb", [128, NBLK * 6], f32,
                          kind="ExternalInput").ap()
    winm = nc.dram_tensor("winm", [128, 6 * NQ], bf,
                          kind="ExternalInput").ap()
    wq = nc.dram_tensor("wq", [128, 128], bf, kind="ExternalInput").ap()
    wk = nc.dram_tensor("wk", [128, 128], bf, kind="ExternalInput").ap()
    wv = nc.dram_tensor("wv", [128, 128], bf, kind="ExternalInput").ap()
    wp = nc.dram_tensor("wp", [128, 128], bf, kind="ExternalInput").ap()
    out = nc.dram_tensor("out", [128, NBLK * NQ], f32,
                         kind="ExternalOutput").ap()

    with tile.TileContext(nc) as tc:
        with tc.tile_pool(name="cst", bufs=1) as cst, \
             tc.tile_pool(name="big", bufs=1) as big, \
             tc.tile_pool(name="qk", bufs=1) as qkp, \
             tc.tile_pool(name="vt", bufs=2) as vtp, \
             tc.tile_pool(name="att", bufs=2) as attp, \
             tc.tile_pool(name="oev", bufs=3) as oev, \
             tc.tile_pool(name="psL", bufs=1, space="PSUM") as psL, \
             tc.tile_pool(name="psO", bufs=1, space="PSUM") as psO, \
             tc.tile_pool(name="psP", bufs=2, space="PSUM") as psP:

            w_q = cst.tile([128, 128], bf)
            nc.gpsimd.dma_start(out=w_q[:], in_=wq[:])
            w_k = cst.tile([128, 128], bf)
            nc.gpsimd.dma_start(out=w_k[:], in_=wk[:])
            w_v = cst.tile([128, 128], bf)
            nc.gpsimd.dma_start(out=w_v[:], in_=wv[:])
            w_p = cst.tile([128, 128], bf)
            nc.gpsimd.dma_start(out=w_p[:], in_=wp[:])

            X = big.tile([128, NBLK * NK], bf)
            nc.gpsimd.dma_start(out=X[:], in_=xc[:])
            WM = big.tile([128, 6 * NQ], bf)   # win mask, g-major
            nc.gpsimd.dma_start(out=WM[:], in_=winm[:])

            mbias = cst.tile([128, NBLK * 6], f32)
            nc.gpsimd.dma_start(out=mbias[:], in_=mb_i[:])

            pL0 = psL.tile([128, 2048], f32, tag="psL")
            nc.vector.memset(pL0[:], 0.0)

            ones = cst.tile([128, 32], bf)
            nc.vector.memset(ones[:], 1.0)

            # Q and K channel-major projections for all blocks.
            Q = qkp.tile([128, NBLK * NQ], bf)
            K = qkp.tile([128, NBLK * NK], bf)
            for blk in range(NBLK):
                pq = psP.tile([128, 512], f32, tag="psP")
                nc.tensor.matmul(out=pq[:, :NQ], lhsT=w_q[:],
                                 rhs=X[:, blk * NK + 144: blk * NK + 144 + NQ],
                                 start=True, stop=True)
                if blk % 2:
                    nc.scalar.copy(out=Q[:, blk * NQ:(blk + 1) * NQ], in_=pq[:, :NQ])
                else:
                    nc.vector.tensor_copy(Q[:, blk * NQ:(blk + 1) * NQ], pq[:, :NQ])
                for half in range(2):
                    pk = psP.tile([128, 512], f32, tag="psP")
                    sl = slice(blk * NK + half * NQ, blk * NK + (half + 1) * NQ)
                    nc.tensor.matmul(out=pk[:, :NQ], lhsT=w_k[:], rhs=X[:, sl],
                                     start=True, stop=True)
                    if half:
                        nc.scalar.copy(out=K[:, sl], in_=pk[:, :NQ])
                    else:
                        nc.vector.tensor_copy(K[:, sl], pk[:, :NQ])

            for blk in range(NBLK):
                # --- V^T production: 6 chunks of 96 pixels ---
                VT = vtp.tile([128, 6 * 128], bf, tag="vt")
                for pair in range(3):       # two 96-chunks per psum bank
                    pv = psP.tile([128, 512], f32, tag="psP")
                    for k2 in range(2):
                        g = pair * 2 + k2
                        nc.tensor.matmul(
                            out=pv[:96, k2 * 128:(k2 + 1) * 128],
                            lhsT=X[:, blk * NK + 96 * g:
                                   blk * NK + 96 * (g + 1)],
                            rhs=w_v[:], start=True, stop=True)
                    if pair == 1:
                        nc.scalar.copy(out=VT[:96, 256:512], in_=pv[:96, :256])
                    else:
                        nc.vector.tensor_copy(
                            VT[:96, pair * 256:(pair + 1) * 256],
                            pv[:96, :256])

                # --- phase 1 + exp + window mask ---
                attnT = attp.tile([128, 4 * 6 * NQ], bf, tag="att")
                for g in range(6):
                    lo, hi = _band(g)
                    nlo, nn = lo * W2, (hi - lo + 1) * W2
                    pL = psL.tile([128, 2048], f32, tag="psL")
                    for h in range(4):
                        nc.tensor.matmul(
                            out=pL[0:96, 512 * h + nlo: 512 * h + nlo + nn],
                            lhsT=K[32 * h:32 * h + 32,
                                   blk * NK + 96 * g:
                                   blk * NK + 96 * g + 96],
                            rhs=Q[32 * h:32 * h + 32,
                                  blk * NQ + nlo:
                                  blk * NQ + nlo + nn],
                            start=True, stop=True,
                            tile_position=(32 * h, 0),
                        )
                    # exp over 4 heads at once: AP [96, (4 banks, nn)]
                    src = pL[:96].rearrange("p (h n) -> p h n", h=4)[:, :, nlo:nlo + nn]
                    dst = attnT[:96].rearrange("p (h g n) -> p h g n", h=4, g=6)[:, :, g, nlo:nlo + nn]
                    nc.scalar.activation(
                        out=dst, in_=src,
                        func=mybir.ActivationFunctionType.Exp,
                        bias=mbias[0:96, blk * 6 + g: blk * 6 + g + 1],
                        scale=SCALE,
                    )
                    # zero out-of-window pairs (win==0) and garbage rows
                    wsrc = WM[0:96, g * NQ + nlo: g * NQ + nlo + nn]
                    for h in range(4):
                        dsth = attnT[0:96, (h * 6 + g) * NQ + nlo:
                                     (h * 6 + g) * NQ + nlo + nn]
                        nc.vector.tensor_mul(out=dsth, in0=dsth, in1=wsrc)

                # --- phase 2 (attn @ V^T) + rowsum, col-tiled by head ---
                pO = psO.tile([128, 512], f32, tag="psO")
                pS = psO.tile([128, 512], f32, tag="psS")
                for g in range(6):
                    lo, hi = _band(g)
                    nlo, nn = lo * W2, (hi - lo + 1) * W2
                    for h in range(4):
                        rhs = attnT[0:96, (h * 6 + g) * NQ + nlo:
                                    (h * 6 + g) * NQ + nlo + nn]
                        nc.tensor.matmul(
                            out=pO[32 * h:32 * h + 32, nlo:nlo + nn],
                            lhsT=VT[0:96, g * 128 + 32 * h:
                                    g * 128 + 32 * h + 32],
                            rhs=rhs, start=(g == 0), stop=(g == 5),
                            tile_position=(0, 32 * h),
                        )
                        nc.tensor.matmul(
                            out=pS[32 * h:32 * h + 32, nlo:nlo + nn],
                            lhsT=ones[0:96, :],
                            rhs=rhs, start=(g == 0), stop=(g == 5),
                            tile_position=(0, 32 * h),
                        )
                rcp = oev.tile([128, NQ], f32, tag="rcp")
                nc.vector.reciprocal(out=rcp[:], in_=pS[:, :NQ])
                onrm = oev.tile([128, NQ], bf, tag="onrm")
                nc.vector.tensor_mul(out=onrm[:], in0=pO[:, :NQ], in1=rcp[:])

                # --- final projection ---
                pF = psP.tile([128, 512], f32, tag="psP")
                nc.tensor.matmul(out=pF[:, :NQ], lhsT=w_p[:], rhs=onrm[:],
                                 start=True, stop=True)
                osb = oev.tile([128, NQ], f32, tag="osb")
                nc.scalar.copy(out=osb[:], in_=pF[:, :NQ])
                nc.gpsimd.dma_start(out=out[:, blk * NQ:(blk + 1) * NQ],
                                    in_=osb[:])

    _split_multi_waits(nc)
    return nc


def _split_multi_waits(nc):
    """This walrus build rejects >1 sem wait per instruction: move extra
    waits onto dedicated single-wait NoOps inserted just before."""
    import copy
    from concourse import mybir

    tmpl = nc.sync.nop(nofuse=True, hint="wsplit_template").ins
    bb0 = nc.cur_bb.bb
    bb0.instructions = [i for i in bb0.instructions if i.name != tmpl.name]
    tmpl = copy.deepcopy(tmpl)

    ctr = 0
    for f in nc.m.functions:
        for bb in f.blocks:
            insts = list(bb.instructions)
            new, changed = [], False
            for inst in insts:
                si = getattr(inst, "sync_info", None)
                waits = list(si.on_wait) if si is not None and si.on_wait else []
                if len(waits) > 1:
                    for w in waits[:-1]:
                        ctr += 1
                        nop = copy.deepcopy(tmpl)
                        nop.name = f"I-wsplit{ctr}"
                        nop.engine = inst.engine
                        nop.sync_info = mybir.SyncInfo(on_wait=[w], on_update=[])
                        new.append(nop)
                    si.on_wait = [waits[-1]]
                    changed = True
                new.append(inst)
            if changed:
                bb.instructions = new


def _host_prep(x, m):
    import ml_dtypes
    bfd = ml_dtypes.bfloat16
    xs, ms = [], []
    for k in range(CORES):
        r0 = 12 * k - 6
        xpad = np.zeros((B, C, 24, W), np.float32)
        mpad = np.zeros((B, 1, 24, W), np.int32)
        lo, hi = max(0, r0), min(H, r0 + 24)
        xpad[:, :, lo - r0:hi - r0] = x[:, :, lo:hi]
        mpad[:, :, lo - r0:hi - r0] = m[:, :, lo:hi]
        xcs = xpad.reshape(B, C, KR, 2, W2, 2).transpose(1, 0, 3, 5, 2, 4)
        xcs = np.ascontiguousarray(xcs.reshape(C, NBLK * NK).astype(bfd))
        mc = mpad.reshape(B, 1, KR, 2, W2, 2).transpose(1, 0, 3, 5, 2, 4)
        mc = mc.reshape(B, 4, NK)
        mb = np.zeros((128, NBLK * 6), np.float32)
        for b in range(B):
            for cspar in range(4):
                for g in range(6):
                    mb[:96, (b * 4 + cspar) * 6 + g] = np.where(
                        mc[b, cspar, 96 * g:96 * (g + 1)] > 0, 0.0, MBIAS)
        xs.append(xcs)
        ms.append(np.ascontiguousarray(mb))
    return xs, ms


def _host_win():
    """[128, 6*NQ] bf16: 0/1 win mask in attnT layout (g-major;
    partitions 96-127 zero)."""
    import ml_dtypes
    win = _win_mask()                        # [NK, NQ]
    wm = np.zeros((128, 6, NQ), np.float32)
    for g in range(6):
        wm[:96, g, :] = win[96 * g:96 * (g + 1), :]
    return np.ascontiguousarray(
        wm.reshape(128, 6 * NQ).astype(ml_dtypes.bfloat16))


def kernel(x, m, Wq, Wk, Wv, Wp):
    global _prog
    import ml_dtypes
    from concourse.bass_utils import run_bass_kernel_spmd

    bfd = ml_dtypes.bfloat16
    x = np.asarray(x, dtype=np.float32)
    m = np.asarray(m, dtype=np.int32)
    if _prog is None:
        _prog = _build_program()
    nc = _prog

    xs, ms = _host_prep(x, m)
    wmask = _host_win()
    base = {
        "winm": wmask,
        "wq": np.ascontiguousarray(np.asarray(Wq, np.float32).T.astype(bfd)),
        "wk": np.ascontiguousarray(np.asarray(Wk, np.float32).T.astype(bfd)),
        "wv": np.ascontiguousarray(np.asarray(Wv, np.float32).T.astype(bfd)),
        "wp": np.ascontiguousarray(np.asarray(Wp, np.float32).T.astype(bfd)),
    }
    in_maps = [{**base, "xc": xs[k], "mb": ms[k]} for k in range(CORES)]
    res = run_bass_kernel_spmd(nc, in_maps, list(range(CORES)))

    full = np.zeros((B, C, H, W), np.float32)
    for k in range(CORES):
        oc = res.results[k]["out"].reshape(C, B, 2, 2, CR, W2)
        o = oc.transpose(1, 0, 4, 2, 5, 3).reshape(B, C, 12, 96)
        full[:, :, 12 * k:12 * k + 12, :] = o
    return full
